# revision 31
# baseline (speedup 1.0000x reference)
"""Sliding-window GQA attention on 8 trn2 NeuronCores.

Sharding: 8 cores = 2 batches x 4 KV groups. Core c=(4*b+g) handles batch b
and query heads [4g, 4g+4) (which share kv head g). Each core computes a
partial output x_b-attention-wo_g; the host sums the 4 group partials per
batch (the wo contraction is split across groups).

Dtypes: projections run fp16 (x, wq/wk/wv converted on host); the wo
contraction runs fp32r on fp32 O^T. The attention core (S, P, P@V) is fp16
(full PE rate at any N). V is projected directly into natural [token, d]
layout — the xT slab is the stationary operand — so there is no vT buffer
and no V transpose at all; the freed PSUM bank deepens the shared
accumulator pool (psX bufs=3).

Schedule: projection chunks (512 tokens) interleave with attention blocks —
after chunk n, query blocks 4n..4n+3 have all their keys, so 4 attention
steps run between chunks and the PE never starves on the x DMA or on the
softmax (Act/DVE) chains. The P^T XBAR for block i is issued in step i+1
and consumed by P@V in step i+2, keeping DMA latency off the critical path.

st=True (default) selects the S^T-oriented attention core: kT chunks are
the stationary operand (per-cell sT[k,q], no P transposes); lsum comes from
a ones-matvec sharing the eT stationary load with the P@V matmul; the
softmax normalize runs on 128-wide O rows (q on partitions) and only 4
O-transposes/block remain. HW-verified at rel 1.692e-2; models 201.4us vs
208.3us for the P-transpose path (st=False). NB: PSUM accumulation regions
sharing a bank must each complete before another region's start=True
(bank-granular pending-zero) — hence the h-outer/c-inner loop in
attn_back_st.
"""
import os
import sys

sys.path.insert(0, "/opt/trn_rl_repo")

import numpy as np

import concourse.bass as bass
import concourse.tile as tile
from concourse import bacc, mybir
from concourse.bass_utils import run_bass_kernel_spmd

B, L, DIM = 2, 2048, 2048
NH, NKV, HD = 16, 4, 128
W = 512
NHL = 4          # query heads per core
GDIM = NHL * HD  # 512 head-dims per core
SCALE = float(HD) ** -0.5
MASKVAL = -60000.0
KC = DIM // 128  # contraction chunks for projections
NB = L // 128    # query blocks
TOK = 512        # token chunk (N of projection matmuls)
NT = L // TOK    # 4 chunks
KH = KC // 2     # 8 contraction chunks per half slab
F32 = mybir.dt.float32
F32R = mybir.dt.float32r
F16 = mybir.dt.float16

_built = {}
last_results = None


def _key_range(i):
    """Keys needed by query block i: [klo, klo+wk)."""
    if i < 4:
        return 0, 128 * (i + 1)
    return 128 * (i - 4), 640


def _build(reps=1, simsafe=False, dmat="", orow="act", fuse=True, pdt="f16",
           st=True, pp=True, oo=True, xq="scalar", lsv=True, sc=True):
    key = (reps, simsafe, dmat, orow, fuse, pdt, st, pp, oo, xq, lsv, sc)
    PDT = F16 if pdt == "f16" else F32R
    if key in _built:
        return _built[key]

    nc = bacc.Bacc("TRN2", target_bir_lowering=False, debug=False,
                   enable_asserts=False)
    # weights arrive host-permuted to partition-major [p, kc, n] so every
    # weight DMA line is the full per-partition extent (4-32KB contiguous)
    # instead of the 256B-1KB rows of the natural [in, out] layout
    xT = nc.dram_tensor("xT", [DIM, L], PDT, kind="ExternalInput").ap()
    wq = nc.dram_tensor("wq", [128, KC * GDIM], PDT,
                        kind="ExternalInput").ap()
    wk = nc.dram_tensor("wk", [128, KC * HD], PDT, kind="ExternalInput").ap()
    wv = nc.dram_tensor("wv", [128, KC * HD], PDT, kind="ExternalInput").ap()
    wo = nc.dram_tensor("wo", [128, NHL * DIM], F32R,
                        kind="ExternalInput").ap()
    # host-built constants: [ident(128) | up(128) | lo(128) | ones(1)] fp16
    cst = nc.dram_tensor("cst", [128, 385], F16, kind="ExternalInput").ap()
    out = nc.dram_tensor("out", [L, DIM], F32, kind="ExternalOutput").ap()

    lag = 2 if "p" in dmat else 1
    # sc shrinks each S psum tile to exactly one bank (chunk-major [k,4h,q]),
    # freeing a bank for a 4th psX accumulator
    psx_bufs = 4 if ("p" in dmat or sc) else 3
    pss_bufs = 3 if sc else 2

    with tile.TileContext(nc) as tc:
      with tc.tile_pool(name="persist", bufs=1) as pers:
        # --- constants (one DMA from the host-built cst input) ---
        csb = pers.tile([128, 385], F16, tag="cst")
        nc.sync.dma_start(out=csb, in_=cst)
        ident16 = csb[:, 0:128]
        up, lo = csb[:, 128:256], csb[:, 256:384]
        mask = [up, lo]
        ones16 = csb[:, 384:385]

        if sc:
            # triangle masks replicated x4 along h so one N=512 matmul
            # pre-masks a whole chunk-major S tile [k, 4h, q]
            maskw4 = pers.tile([128, 2, NHL, 128], F16, tag="maskw4")
            for w_ in range(2):
                for h in range(NHL):
                    nc.vector.tensor_copy(maskw4[:, w_, h, :], mask[w_])

        # --- persistent tensors ---
        # single qT tile: S reads all four heads' query block as one
        # strided N=512 moving operand
        qT4 = pers.tile([128, NHL, L], F16, tag="qT4", name="qT4")
        qT = [qT4[:, m, :] for m in range(NHL)]
        kT = pers.tile([128, L], F16, tag="kT")
        # lsv: V chunks carry a 129th all-ones column, so the PV matmul's
        # output column 128 is the softmax denominator per query — an N=129
        # matmul costs the same as N=128 and replaces the per-cell lsum
        # matvecs entirely.
        VW = 129 if lsv else 128
        vnat = pers.tile([128, KC, VW], F16, tag="vnat")
        if lsv:
            nc.vector.memset(vnat[:, :, 128:VW], 1.0)

        def mask_mm(dst, which, stop):
            """Add a triangle mask on top of already-written S values in
            PSUM (addition commutes, so mask-after-S == S-after-mask, and
            every PSUM region's first write carries start=True)."""
            nc.tensor.matmul(dst, ident16, mask[which],
                             start=False, stop=stop, skip_group_check=True)

        for _rep in range(reps):
            with tc.tile_pool(name="paw", bufs=1) as paw, \
                 tc.tile_pool(name="pax", bufs=2) as pax, \
                 tc.tile_pool(name="pb", bufs=2) as pb, \
                 tc.tile_pool(name="pco", bufs=2) as pco, \
                 tc.tile_pool(name="psX", bufs=psx_bufs, space="PSUM") as psX, \
                 tc.tile_pool(name="psT", bufs=1, space="PSUM") as psT, \
                 tc.tile_pool(name="psS", bufs=pss_bufs, space="PSUM") as psS:
                wq_sb = paw.tile([128, KC, GDIM], PDT, tag="wq")
                wk_sb = paw.tile([128, KC, HD], PDT, tag="wk")
                wv_sb = paw.tile([128, KC, HD], PDT, tag="wv")
                wo_sb = paw.tile([128, NHL, DIM], F32R, tag="wo")
                wq_r = wq.rearrange("p (kc n) -> p kc n", kc=KC)
                wk_r = wk.rearrange("p (kc n) -> p kc n", kc=KC)
                wv_r = wv.rearrange("p (kc n) -> p kc n", kc=KC)
                xT_r = xT.rearrange("(kc p) t -> p kc t", p=128)

                def load_half(n, half, splits=2):
                    # split-slab DMAs into one half-slab tile so the first
                    # matmuls unblock after a fraction of the data (the very
                    # first slab uses finer splits to cut kernel warmup).
                    # bufs=8 keeps every slab of the rep resident, so
                    # prefetches never WAR-stall on a live consumer.
                    xh = pax.tile([128, KH, TOK], PDT, tag="x", name="xh",
                                  bufs=8 if pp else 2)
                    step = KH // splits
                    for q in range(splits):
                        ks = np.s_[:, half * KH + step * q:
                                   half * KH + step * (q + 1),
                                   n * TOK:(n + 1) * TOK]
                        nc.sync.dma_start(
                            out=xh[:, step * q:step * (q + 1), :],
                            in_=xT_r[ks])
                    return xh

                slabs = {}

                def proj_chunk(n):
                    if n == 0:
                        # first wq quarter + both x halves first so the
                        # m-loop's first matmuls unblock asap; remaining
                        # weights and wo after
                        ksl0 = np.s_[:, 0:4, :]
                        nc.sync.dma_start(out=wq_sb[ksl0], in_=wq_r[ksl0])
                        slabs[(0, 0)] = load_half(0, 0, splits=4)
                        slabs[(0, 1)] = load_half(0, 1)
                        for q4 in range(1, 4):
                            ksl = np.s_[:, 4 * q4:4 * (q4 + 1), :]
                            nc.sync.dma_start(out=wq_sb[ksl], in_=wq_r[ksl])
                        for q4 in range(4):
                            ksl = np.s_[:, 4 * q4:4 * (q4 + 1), :]
                            nc.sync.dma_start(out=wk_sb[ksl], in_=wk_r[ksl])
                            nc.sync.dma_start(out=wv_sb[ksl], in_=wv_r[ksl])
                        nc.sync.dma_start(
                            out=wo_sb,
                            in_=wo.rearrange("p (kc n) -> p kc n", kc=NHL))
                    x_lo = slabs.pop((n, 0))
                    x_hi = slabs.pop((n, 1))
                    for m in range(NHL + 1):
                        acc = psX.tile([128, TOK], F32, tag="acc", name="acc")
                        for kc in range(KC):
                            if m < NHL:
                                lhsT = wq_sb[:, kc, 128 * m:128 * (m + 1)]
                            else:
                                lhsT = wk_sb[:, kc, :]
                            xh = x_lo if kc < KH else x_hi
                            nc.tensor.matmul(acc, lhsT, xh[:, kc % KH, :],
                                             start=(kc == 0),
                                             stop=(kc == KC - 1))
                        sl = np.s_[:, n * TOK:(n + 1) * TOK]
                        if m < NHL:
                            nc.vector.tensor_copy(qT[m][sl], acc)
                        else:
                            nc.vector.tensor_copy(kT[sl], acc)
                        # prefetch next chunk's slabs midway through
                        if m == 0 and n + 1 < NT:
                            slabs[(n + 1, 0)] = load_half(n + 1, 0)
                        if m == 2 and n + 1 < NT:
                            slabs[(n + 1, 1)] = load_half(n + 1, 1)
                    # V directly in natural [token, d] layout: the xT slab is
                    # the stationary operand, wv the moving one — no vT, no
                    # transposes. 4 token-block regions per psum slot,
                    # region-outer (bank-granular pending-zero).
                    vacc = psX.tile([128, TOK], F32, tag="acc", name="vacc")
                    for tb in range(4):
                        for kc in range(KC):
                            xh = x_lo if kc < KH else x_hi
                            nc.tensor.matmul(
                                vacc[:, 128 * tb:128 * (tb + 1)],
                                xh[:, kc % KH, 128 * tb:128 * (tb + 1)],
                                wv_sb[:, kc, :],
                                start=(kc == 0), stop=(kc == KC - 1),
                                skip_group_check=True)
                    nc.vector.tensor_copy(
                        vnat[:, 4 * n:4 * (n + 1), 0:128], vacc)

                phalfs = {}

                def load_phalf(p2, half, splits=2):
                    # one slab covers BOTH chunks of the pair: 2KB
                    # contiguous lines halve the HWDGE descriptor work per
                    # byte vs per-chunk 1KB-line slabs
                    xh = pax.tile([128, KH, 2 * TOK], PDT, tag="x",
                                  name="xh", bufs=4)
                    step = KH // splits
                    for q in range(splits):
                        ks = np.s_[:, half * KH + step * q:
                                   half * KH + step * (q + 1),
                                   2 * p2 * TOK:(2 * p2 + 2) * TOK]
                        nc.sync.dma_start(
                            out=xh[:, step * q:step * (q + 1), :],
                            in_=xT_r[ks])
                    return xh

                def proj_pair(p2):
                    # two token chunks per stationary: each LDWEIGHTS serves
                    # 1024 moving columns instead of 512, hiding the weight
                    # load the cost model ignores (~52ns/MM measured on HW)
                    n0, n1 = 2 * p2, 2 * p2 + 1
                    if p2 == 0:
                        # k-projection runs first, so wk + the first slabs
                        # gate the first matmuls, not the 2MB wq
                        for q4 in range(4):
                            ksl = np.s_[:, 4 * q4:4 * (q4 + 1), :]
                            nc.sync.dma_start(out=wk_sb[ksl], in_=wk_r[ksl])
                        phalfs[(0, 0)] = load_phalf(0, 0, splits=4)
                        phalfs[(0, 1)] = load_phalf(0, 1)
                        for q4 in range(4):
                            ksl = np.s_[:, 4 * q4:4 * (q4 + 1), :]
                            nc.sync.dma_start(out=wq_sb[ksl], in_=wq_r[ksl])
                        for q4 in range(4):
                            ksl = np.s_[:, 4 * q4:4 * (q4 + 1), :]
                            nc.sync.dma_start(out=wv_sb[ksl], in_=wv_r[ksl])
                        nc.sync.dma_start(
                            out=wo_sb,
                            in_=wo.rearrange("p (kc n) -> p kc n", kc=NHL))
                    x_lo = phalfs.pop((p2, 0))
                    x_hi = phalfs.pop((p2, 1))
                    for m in [NHL] + list(range(NHL)):
                        acc0 = psX.tile([128, TOK], F32, tag="acc", name="acc")
                        acc1 = psX.tile([128, TOK], F32, tag="acc", name="acc")
                        for kc in range(KC):
                            if m < NHL:
                                lhsT = wq_sb[:, kc, 128 * m:128 * (m + 1)]
                            else:
                                lhsT = wk_sb[:, kc, :]
                            xh = x_lo if kc < KH else x_hi
                            nc.tensor.matmul(acc0, lhsT,
                                             xh[:, kc % KH, 0:TOK],
                                             start=(kc == 0),
                                             stop=(kc == KC - 1))
                            nc.tensor.matmul(acc1, lhsT,
                                             xh[:, kc % KH, TOK:2 * TOK],
                                             start=(kc == 0),
                                             stop=(kc == KC - 1))
                        for nn, acc in ((n0, acc0), (n1, acc1)):
                            sl = np.s_[:, nn * TOK:(nn + 1) * TOK]
                            if m < NHL:
                                nc.vector.tensor_copy(qT[m][sl], acc)
                            else:
                                nc.vector.tensor_copy(kT[sl], acc)
                        # prefetch the next pair's slabs spread over the loop
                        if p2 == 0 and m in (0, 2):
                            phalfs[(1, m // 2)] = load_phalf(1, m // 2)
                    for cc in range(2):
                        nn = n0 + cc
                        vacc = psX.tile([128, TOK], F32, tag="acc",
                                        name="vacc")
                        for tb in range(4):
                            for kc in range(KC):
                                xh = x_lo if kc < KH else x_hi
                                nc.tensor.matmul(
                                    vacc[:, 128 * tb:128 * (tb + 1)],
                                    xh[:, kc % KH,
                                       TOK * cc + 128 * tb:
                                       TOK * cc + 128 * (tb + 1)],
                                    wv_sb[:, kc, :],
                                    start=(kc == 0), stop=(kc == KC - 1),
                                    skip_group_check=True)
                        nc.vector.tensor_copy(
                            vnat[:, 4 * nn:4 * (nn + 1), 0:128], vacc)

                ptq_saved = {}
                p_saved = {}
                oT_saved = {}

                def emit_ptrans(i):
                    # XBAR transposes for block i's P tile; deferred one
                    # attention step so Act.SEQ never waits on the DVE
                    # normalize chain, and consumed another step later so
                    # the XBAR latency stays off the critical path
                    klo, wkk = _key_range(i)
                    nch = wkk // 128
                    p_all = p_saved.pop(i)
                    ptq = pb.tile([128, NHL, 5, 128], F16, tag="ptq",
                                  name="ptq")
                    xeng = nc.sync if xq == "sync" else nc.scalar
                    if nch == 5:
                        # all heads + chunks in one XBAR instruction:
                        # in free index h*640 + c*128 + k -> out[:, h, c, :]
                        xeng.dma_start(out=ptq, in_=p_all, transpose=True)
                    else:
                        for h in range(NHL):
                            xeng.dma_start(out=ptq[:, h, 0:nch, :],
                                           in_=p_all[:, h, :wkk],
                                           transpose=True)
                    ptq_saved[i] = ptq

                eT_saved = {}
                ls_saved = {}

                def attn_front_st(i):
                    # S^T orientation: sT[k, c, q] per head; exp -> eT fp16;
                    # no P transposes (O is transposed instead, 4/block)
                    klo, wkk = _key_range(i)
                    nch = wkk // 128
                    if sc:
                        # chunk-major: one S tile [k, 4h, q] per key chunk
                        # (exactly one psum bank). The kT chunk stationary is
                        # loaded once for all four heads, and masked chunks
                        # take a single wide N=512 mask matmul written first
                        # (per-element has_written: the S matmuls accumulate
                        # onto it).
                        eTcs = [None] * nch
                        # masked chunks first so their two wide mask matmuls
                        # share one ident LDWEIGHTS; kT stationaries follow
                        order = sorted(
                            range(nch),
                            key=lambda c: 0 if (klo + 128 * c == 128 * i or
                                                (i >= 4 and c == 0)) else 1)
                        tiles, started = {}, {}
                        for c in order:
                            kg = klo + 128 * c
                            diag = kg == 128 * i
                            edge = i >= 4 and c == 0
                            masked = diag or edge
                            st_c = psS.tile([128, NHL, 128], F32, tag="s",
                                            name="st_c")
                            tiles[c], started[c] = st_c, masked
                            if masked:
                                nc.tensor.matmul(
                                    st_c, ident16,
                                    maskw4[:, 0 if diag else 1],
                                    start=True, stop=False,
                                    skip_group_check=True)
                        for c in order:
                            st_c = tiles[c]
                            kg = klo + 128 * c
                            nc.tensor.matmul(
                                st_c, kT[:, kg:kg + 128],
                                qT4[:, :, 128 * i:128 * (i + 1)],
                                start=not started[c],
                                stop=True, skip_group_check=True)
                            eTc = pb.tile([128, NHL, 128], F16, tag="eTc",
                                          name="eTc", bufs=12)
                            nc.scalar.activation(
                                out=eTc, in_=st_c,
                                func=mybir.ActivationFunctionType.Exp,
                                scale=SCALE)
                            eTcs[c] = eTc
                        eT_saved[i] = eTcs
                        return
                    eTs = []
                    for h in range(NHL):
                        st_ps = psS.tile([128, 5, 128], F32, tag="s",
                                         name="st_ps")
                        for c in range(nch):
                            kg = klo + 128 * c
                            diag = kg == 128 * i
                            edge = i >= 4 and c == 0
                            nc.tensor.matmul(
                                st_ps[:, c, :], kT[:, kg:kg + 128],
                                qT[h][:, 128 * i:128 * (i + 1)],
                                start=True, stop=not (diag or edge),
                                skip_group_check=True)
                            if diag:
                                # diagonal cell: invalid where k > q
                                mask_mm(st_ps[:, c, :], 0, stop=True)
                            elif edge:
                                # window edge: invalid where q_l > k_l
                                mask_mm(st_ps[:, c, :], 1, stop=True)
                        eT = pb.tile([128, 5, 128], F16, tag=f"eT{h}",
                                     name="eT", bufs=2)
                        nc.scalar.activation(
                            out=eT[:, 0:nch, :], in_=st_ps[:, 0:nch, :],
                            func=mybir.ActivationFunctionType.Exp,
                            scale=SCALE)
                        eTs.append(eT)
                    eT_saved[i] = eTs

                def attn_back_st(i):
                    klo, wkk = _key_range(i)
                    nch = wkk // 128
                    eTs = eT_saved.pop(i)
                    if sc:
                        cell = lambda h, c: eTs[c][:, h, :]
                    else:
                        cell = lambda h, c: eTs[h][:, c, :]
                    linv = pb.tile([128, 4], F32, tag="linv2", name="linv",
                                   bufs=2)
                    if lsv:
                        # two banks x two heads of [q, 129]: col 128 is the
                        # denominator (ones-column of vnat), so there are no
                        # separate lsum matvecs at all
                        o_a = psX.tile([128, 2, VW], F32, tag="acc",
                                       name="o_a")
                        o_b = psX.tile([128, 2, VW], F32, tag="acc",
                                       name="o_b")
                        obank = [o_a, o_a, o_b, o_b]
                        for h in range(NHL):
                            # h outer: regions in a shared bank complete
                            # before the next region's start=True
                            dst = obank[h][:, h % 2, :]
                            for c in range(nch):
                                nc.tensor.matmul(
                                    dst, cell(h, c),
                                    vnat[:, klo // 128 + c, :],
                                    start=(c == 0), stop=(c == nch - 1),
                                    skip_group_check=True)
                        nc.vector.reciprocal(linv[:, 0:2], o_a[:, :, 128])
                        nc.vector.reciprocal(linv[:, 2:4], o_b[:, :, 128])
                        o_n = pb.tile([128, NHL, 128], F16, tag="on",
                                      name="o_n", bufs=2)
                        for h in range(NHL):
                            nc.vector.tensor_scalar_mul(
                                o_n[:, h, :], obank[h][:, h % 2, 0:128],
                                linv[:, h:h + 1])
                    else:
                        o_ps = psX.tile([128, 512], F32, tag="acc",
                                        name="o_ps")
                        if oo:
                            # ls lives in the psT bank (sequential with
                            # ot_ps), freeing a psX slot for the paired
                            # out_tile accs
                            ls_ps = psT.tile([128, 512], F32, tag="t",
                                             name="ls_ps")
                        else:
                            ls_ps = psX.tile([128, 512], F32, tag="acc",
                                             name="ls_ps")
                        for h in range(NHL):
                            # h outer: each PSUM region's accumulation
                            # completes before another region's start=True
                            # marks the bank pending-zero. eT cell is the
                            # stationary operand for both the PV matmul and
                            # the lsum matvec (one load).
                            for c in range(nch):
                                nc.tensor.matmul(
                                    o_ps[:, 128 * h:128 * (h + 1)],
                                    cell(h, c),
                                    vnat[:, klo // 128 + c, 0:128],
                                    start=(c == 0), stop=(c == nch - 1),
                                    skip_group_check=True)
                                nc.tensor.matmul(
                                    ls_ps[:, h:h + 1], cell(h, c),
                                    ones16,
                                    start=(c == 0), stop=(c == nch - 1),
                                    skip_group_check=True)
                        nc.vector.reciprocal(linv, ls_ps[:, 0:4])
                        o_n = pb.tile([128, NHL, 128], F16, tag="on",
                                      name="o_n", bufs=2)
                        for h in range(NHL):
                            nc.vector.tensor_scalar_mul(
                                o_n[:, h, :], o_ps[:, 128 * h:128 * (h + 1)],
                                linv[:, h:h + 1])
                    # O^T via 4 PE transposes into one bank, one copy out
                    ot_ps = psT.tile([128, 512], F16, tag="t", name="ot_ps")
                    for h in range(NHL):
                        nc.tensor.transpose(ot_ps[:, 128 * h:128 * (h + 1)],
                                            o_n[:, h, :], ident16)
                    oT = pb.tile([128, NHL, 128], F32R, tag="oT", name="oT",
                                 bufs=4)
                    nc.vector.tensor_copy(oT, ot_ps)
                    oT_saved[i] = oT

                def attn_front(i):
                    klo, wkk = _key_range(i)
                    nch = wkk // 128
                    if i >= 1 and "p" in dmat:
                        emit_ptrans(i - 1)
                    p_all = pb.tile([128, NHL, 640], F16, tag="p",
                                    name="p_all")
                    for h in range(NHL):
                        s_ps = psS.tile([128, 640], F32, tag="s", name="s_ps")
                        if i < 4:
                            # S with start=True, then the causal triangle
                            # added on the last 128 cols
                            nc.tensor.matmul(
                                s_ps[:, 0:wkk],
                                qT[h][:, 128 * i:128 * (i + 1)],
                                kT[:, klo:klo + wkk],
                                start=True, stop=False, skip_group_check=True)
                            mask_mm(s_ps[:, wkk - 128:wkk], 1, stop=True)
                        else:
                            nc.tensor.matmul(
                                s_ps[:, 0:512],
                                qT[h][:, 128 * i:128 * (i + 1)],
                                kT[:, klo:klo + 512],
                                start=True, stop=False, skip_group_check=True)
                            mask_mm(s_ps[:, 0:128], 0, stop=False)
                            nc.tensor.matmul(
                                s_ps[:, 512:640],
                                qT[h][:, 128 * i:128 * (i + 1)],
                                kT[:, klo + 512:klo + 640],
                                start=True, stop=False, skip_group_check=True)
                            mask_mm(s_ps[:, 512:640], 1, stop=True)
                        e_sb = pb.tile([128, 640], F32, tag="e",
                                       name="e_sb", bufs=4)
                        lsum = pb.tile([128, 1], F32, tag=f"l{h}", name="lsum",
                                       bufs=2)
                        nc.scalar.activation(
                            out=e_sb[:, :wkk], in_=s_ps[:, :wkk],
                            func=mybir.ActivationFunctionType.Exp,
                            scale=SCALE, accum_out=lsum)
                        linv = pb.tile([128, 1], F32, tag=f"li{h}",
                                       name="linv", bufs=2)
                        nc.vector.reciprocal(linv, lsum)
                        # NB: DVE, not gpsimd — gpsimd tensor_scalar costs
                        # ~7us/op on HW and serializes the block pipeline
                        nc.vector.tensor_scalar_mul(p_all[:, h, :wkk],
                                                    e_sb[:, :wkk], linv)
                    p_saved[i] = p_all
                    if "p" not in dmat:
                        ptq = pb.tile([128, NHL, 5, 128], F16, tag="ptq",
                                      name="ptq")
                        for c in range(nch):
                            t_ps = psT.tile([128, 512], F16, tag="t",
                                            name="t_ps")
                            for h in range(NHL):
                                nc.tensor.transpose(
                                    t_ps[:, 128 * h:128 * (h + 1)],
                                    p_all[:, h, 128 * c:128 * (c + 1)],
                                    ident16)
                            nc.vector.tensor_copy(ptq[:, :, c, :], t_ps)
                        p_saved.pop(i, None)
                        ptq_saved[i] = ptq

                def attn_back(i):
                    klo, wkk = _key_range(i)
                    nch = wkk // 128
                    ptq = ptq_saved.pop(i)
                    # O^T accumulation over key chunks (all heads at once)
                    o_ps = psX.tile([128, 512], F32, tag="acc", name="o_ps")
                    for c in range(nch):
                        nc.tensor.matmul(
                            o_ps, vnat[:, klo // 128 + c, 0:128],
                            ptq[:, :, c, :],
                            start=(c == 0), stop=(c == nch - 1))
                    # rolling O^T buffer: consumed by out_tile 3 steps later
                    oT = pb.tile([128, NHL, 128], F32R, tag="oT", name="oT",
                                 bufs=4)
                    nc.vector.tensor_copy(oT, o_ps)
                    oT_saved[i] = oT

                def out_tile(tt):
                    oT = oT_saved.pop(tt)
                    if oo and sc:
                        # 4-way: each oT[kc] stationary serves all 2048 wo
                        # columns (4 psum banks), and the out DMA is one
                        # 8KB-per-partition-line transfer. The last block
                        # tapers: per-quarter copies+DMAs so the kernel tail
                        # is one quarter's copy+store, not the whole row's.
                        o_row = pco.tile([128, DIM], F32, tag="orow4",
                                         name="o_row")
                        accs = [psX.tile([128, 512], F32, tag="acc",
                                         name="acc") for _ in range(4)]
                        for kc in range(NHL):
                            for nn in range(4):
                                nc.tensor.matmul(
                                    accs[nn], oT[:, kc, :],
                                    wo_sb[:, kc, 512 * nn:512 * (nn + 1)],
                                    start=(kc == 0), stop=(kc == NHL - 1))
                        taper = tt >= NB - 2
                        for nn in range(4):
                            nc.scalar.activation(
                                out=o_row[:, 512 * nn:512 * (nn + 1)],
                                in_=accs[nn],
                                func=mybir.ActivationFunctionType.Copy)
                            if taper:
                                nc.sync.dma_start(
                                    out=out[128 * tt:128 * (tt + 1),
                                            512 * nn:512 * (nn + 1)],
                                    in_=o_row[:, 512 * nn:512 * (nn + 1)])
                        if not taper:
                            nc.sync.dma_start(
                                out=out[128 * tt:128 * (tt + 1), :],
                                in_=o_row)
                        return
                    for half in range(2):
                        o_row = pco.tile([128, DIM // 2], F32, tag="orow",
                                         name="o_row")
                        if oo:
                            # kc-outer with two psum banks: each oT[kc]
                            # stationary serves 1024 moving wo columns
                            acc0 = psX.tile([128, 512], F32, tag="acc",
                                            name="acc")
                            acc1 = psX.tile([128, 512], F32, tag="acc",
                                            name="acc")
                            base = 1024 * half
                            for kc in range(NHL):
                                nc.tensor.matmul(
                                    acc0, oT[:, kc, :],
                                    wo_sb[:, kc, base:base + 512],
                                    start=(kc == 0), stop=(kc == NHL - 1))
                                nc.tensor.matmul(
                                    acc1, oT[:, kc, :],
                                    wo_sb[:, kc, base + 512:base + 1024],
                                    start=(kc == 0), stop=(kc == NHL - 1))
                            for sub, acc in ((0, acc0), (1, acc1)):
                                nc.scalar.activation(
                                    out=o_row[:, 512 * sub:512 * (sub + 1)],
                                    in_=acc,
                                    func=mybir.ActivationFunctionType.Copy)
                        else:
                            for sub in range(2):
                                nn = 2 * half + sub
                                acc = psX.tile([128, 512], F32, tag="acc",
                                               name="acc")
                                for kc in range(NHL):
                                    nc.tensor.matmul(
                                        acc, oT[:, kc, :],
                                        wo_sb[:, kc, 512 * nn:512 * (nn + 1)],
                                        start=(kc == 0), stop=(kc == NHL - 1))
                                on_act = (orow == "act" or
                                          (orow == "mix" and nn % 2 == 1))
                                if on_act:
                                    nc.scalar.activation(
                                        out=o_row[:, 512 * sub:512 * (sub + 1)],
                                        in_=acc,
                                        func=mybir.ActivationFunctionType.Copy)
                                else:
                                    nc.vector.tensor_copy(
                                        o_row[:, 512 * sub:512 * (sub + 1)],
                                        acc)
                        nc.sync.dma_start(
                            out=out[128 * tt:128 * (tt + 1),
                                    1024 * half:1024 * (half + 1)],
                            in_=o_row)

                def att_step(i):
                    (attn_front_st if st else attn_front)(i)
                    if i >= lag:
                        (attn_back_st if st else attn_back)(i - lag)
                    if i >= lag + 1:
                        out_tile(i - lag - 1)

                if fuse and pp:
                    for p2 in range(NT // 2):
                        proj_pair(p2)
                        for i in range(8 * p2, 8 * (p2 + 1)):
                            att_step(i)
                elif fuse:
                    for n in range(NT):
                        proj_chunk(n)
                        for i in range(4 * n, 4 * (n + 1)):
                            att_step(i)
                else:
                    for n in range(NT):
                        proj_chunk(n)
                    for i in range(NB):
                        att_step(i)
                # drain
                if "p" in dmat and not st:
                    emit_ptrans(NB - 1)
                for i in range(NB - lag, NB):
                    (attn_back_st if st else attn_back)(i)
                for tt in range(NB - lag - 1, NB):
                    out_tile(tt)

    nc.compile()
    _built[key] = nc
    return nc


def _pmajor(w):
    """[kc*128, n] -> partition-major [128, kc*n] (flattened per p)."""
    kc = w.shape[0] // 128
    n = w.shape[1]
    return np.ascontiguousarray(
        w.reshape(kc, 128, n).transpose(1, 0, 2).reshape(128, kc * n))


def _make_consts():
    """[ident(128) | up(128) | lo(128) | ones(1)] fp16, matching the old
    gpsimd affine_select patterns."""
    cst = np.zeros((128, 385), np.float16)
    p = np.arange(128)[:, None]
    j = np.arange(128)[None, :]
    cst[:, 0:128] = (j == p).astype(np.float16)        # identity
    cst[:, 128:256] = np.where(j >= p, 0.0, MASKVAL)   # 'up': mask k > q
    cst[:, 256:384] = np.where(p >= j, 0.0, MASKVAL)   # 'lo': mask q_l > k_l
    cst[:, 384] = 1.0
    return cst


def prep_inputs(x, wq, wk, wv, wo, pdt="f16"):
    """Full fp32 inputs -> per-core input maps (8 cores)."""
    npdt = np.float16 if pdt == "f16" else np.float32
    x = np.asarray(x, dtype=np.float32)
    xT = [np.ascontiguousarray(x[b].T.astype(npdt)) for b in range(B)]
    wqh = np.asarray(wq, dtype=npdt)
    wkh = np.asarray(wk, dtype=npdt)
    wvh = np.asarray(wv, dtype=npdt)
    woh = np.asarray(wo, dtype=np.float32)
    cst = _make_consts()
    in_maps = []
    for c in range(8):
        b, g = c // 4, c % 4
        in_maps.append({
            "xT": xT[b],
            "wq": _pmajor(wqh[:, GDIM * g:GDIM * (g + 1)]),
            "wk": _pmajor(wkh[:, HD * g:HD * (g + 1)]),
            "wv": _pmajor(wvh[:, HD * g:HD * (g + 1)]),
            "wo": _pmajor(woh[GDIM * g:GDIM * (g + 1), :]),
            "cst": cst,
        })
    return in_maps


def kernel(x, wq, wk, wv, wo):
    global last_results
    nc = _build()
    in_maps = prep_inputs(x, wq, wk, wv, wo)
    res = run_bass_kernel_spmd(nc, in_maps, list(range(8)))
    last_results = res
    out = np.empty((B, L, DIM), dtype=np.float32)
    for b in range(B):
        acc = np.zeros((L, DIM), dtype=np.float64)
        for g in range(4):
            acc += res.results[4 * b + g]["out"]
        out[b] = acc.astype(np.float32)
    return out

